# revision 1
# baseline (speedup 1.0000x reference)
"""Trainium2 Bass kernel for MinimalLightningIndexer.

out[b,t,s] = relu((x@Wq)[b,t] . (x@Wk)[b,s]) * (x@Ww)[b,t]

Sharding: 8 cores = 4 batches x 2 query-halves. Each core computes the
[2048, 4096] score block for its (batch, t-half). The host feeds each
core x[b].T (d-major, contiguous) with the core's own t-half tokens
permuted to the front, so one SPMD program serves all cores; the host
un-permutes score columns when assembling the full output.

Per-core device program:
  - load x.T slabs [2048d x 512tok] (4 MB DMAs, natural layout)
  - PE: kT[16,512] per token chunk (all 8), qT/wT[17,512] (own 4 chunks),
    f32 matmuls accumulating over 16 d-chunks of 128
  - one SBUF->SBUF DMA transposes wT[1,2048] -> w_col[128,16]
  - scores: matmul qT_tile.T @ kT chunk (K=16, N=512) -> PSUM,
    ScalarE relu PSUM->SBUF, VectorE per-partition gate multiply,
    1 MB output DMAs
"""

import sys

if "/opt/trn_rl_repo" not in sys.path:
    sys.path.insert(0, "/opt/trn_rl_repo")

import numpy as np

import concourse.bacc as bacc
import concourse.bass as bass
import concourse.mybir as mybir
import concourse.tile as tile
from concourse.bass_utils import run_bass_kernel_spmd

B, S, D = 4, 4096, 2048
IDX = 16
N_CORES = 8
T = S // 2          # query tokens per core
DC = D // 128       # 16 d-chunks
SC = S // 512       # 8 token chunks
TC = T // 512       # 4 own token chunks
TT = T // 128       # 16 t-tiles

_CACHE = {}


def _build_nc():
    if "nc" in _CACHE:
        return _CACHE["nc"]
    f32 = mybir.dt.float32
    bf16 = mybir.dt.bfloat16
    nc = bacc.Bacc("TRN2", target_bir_lowering=False, debug=False,
                   num_devices=N_CORES)
    xt = nc.dram_tensor("xt", [D, S], bf16, kind="ExternalInput").ap()
    wk = nc.dram_tensor("wk", [D, IDX], bf16, kind="ExternalInput").ap()
    wqw = nc.dram_tensor("wqw", [D, IDX + 1], bf16, kind="ExternalInput").ap()
    o = nc.dram_tensor("o", [T, S], bf16, kind="ExternalOutput").ap()

    with tile.TileContext(nc) as tc:
        with (
            tc.tile_pool(name="const", bufs=1) as cpool,
            tc.tile_pool(name="slab", bufs=3) as slab_pool,
            tc.tile_pool(name="osb", bufs=4) as out_pool,
            tc.tile_pool(name="pk", bufs=2, space="PSUM") as pk_pool,
            tc.tile_pool(name="pqw", bufs=2, space="PSUM") as pqw_pool,
            tc.tile_pool(name="ps", bufs=4, space="PSUM") as ps_pool,
        ):
            # --- persistent small tensors ---
            wk_sb = cpool.tile([128, DC * IDX], bf16, tag="wk_sb")
            nc.sync.dma_start(
                out=wk_sb[:],
                in_=wk.rearrange("(kd p) i -> p kd i", p=128),
            )
            wqw_sb = cpool.tile([128, DC * (IDX + 1)], bf16, tag="wqw_sb")
            nc.sync.dma_start(
                out=wqw_sb[:],
                in_=wqw.rearrange("(kd p) i -> p kd i", p=128),
            )
            kt_sb = cpool.tile([IDX, S], bf16, tag="kt_sb")
            qw_sb = cpool.tile([IDX + 1, T], bf16, tag="qw_sb")
            qwf_sb = cpool.tile([IDX + 1, T], f32, tag="qwf_sb")
            w_col = cpool.tile([128, TT], f32, tag="w_col")

            # --- projections per 512-token chunk ---
            for j in range(SC):
                slab = slab_pool.tile([128, DC * 512], bf16, tag="slab")
                nc.sync.dma_start(
                    out=slab[:],
                    in_=xt[:, j * 512:(j + 1) * 512].rearrange(
                        "(kd p) s -> p kd s", p=128),
                )
                slab_v = slab[:].rearrange("p (kd t) -> p kd t", kd=DC)

                psk = pk_pool.tile([IDX, 512], f32, tag="psk")
                for kd in range(DC):
                    nc.tensor.matmul(
                        psk[:],
                        wk_sb[:, kd * IDX:(kd + 1) * IDX],
                        slab_v[:, kd, :],
                        start=(kd == 0), stop=(kd == DC - 1),
                    )
                nc.vector.tensor_copy(kt_sb[:, j * 512:(j + 1) * 512], psk[:])

                if j < TC:
                    psqw = pqw_pool.tile([IDX + 1, 512], f32, tag="psqw")
                    for kd in range(DC):
                        nc.tensor.matmul(
                            psqw[:],
                            wqw_sb[:, kd * (IDX + 1):(kd + 1) * (IDX + 1)],
                            slab_v[:, kd, :],
                            start=(kd == 0), stop=(kd == DC - 1),
                        )
                    nc.vector.tensor_copy(
                        qw_sb[:, j * 512:(j + 1) * 512], psqw[:])
                    nc.vector.tensor_copy(
                        qwf_sb[:, j * 512:(j + 1) * 512], psqw[:])

            # --- transpose gate row wT[1, T] -> w_col[128, TT] ---
            for i in range(TT):
                nc.sync.dma_start(
                    out=w_col[:, i:i + 1],
                    in_=qwf_sb[IDX:IDX + 1, i * 128:(i + 1) * 128],
                )

            # --- scores ---
            for i in range(TT):
                for jq in range(2):
                    osb = out_pool.tile([128, 2048], bf16, tag="osb")
                    for jj in range(4):
                        j = jq * 4 + jj
                        pss = ps_pool.tile([128, 512], f32, tag="pss")
                        nc.tensor.matmul(
                            pss[:],
                            qw_sb[0:IDX, i * 128:(i + 1) * 128],
                            kt_sb[:, j * 512:(j + 1) * 512],
                            start=True, stop=True,
                        )
                        nc.scalar.activation(
                            osb[:, jj * 512:(jj + 1) * 512], pss[:],
                            mybir.ActivationFunctionType.Relu,
                        )
                        nc.vector.tensor_scalar_mul(
                            out=osb[:, jj * 512:(jj + 1) * 512],
                            in0=osb[:, jj * 512:(jj + 1) * 512],
                            scalar1=w_col[:, i:i + 1],
                        )
                    nc.sync.dma_start(
                        out=o[i * 128:(i + 1) * 128,
                              jq * 2048:(jq + 1) * 2048],
                        in_=osb[:],
                    )
    nc.compile()
    _CACHE["nc"] = nc
    return nc


def _make_in_maps(x, Wq, Wk, Ww):
    import ml_dtypes
    bf = ml_dtypes.bfloat16
    wqw = np.ascontiguousarray(
        np.concatenate([Wq, Ww], axis=1)).astype(bf)
    wk = np.ascontiguousarray(Wk).astype(bf)
    xbf = x.astype(bf)
    in_maps = []
    for c in range(N_CORES):
        b, h = c // 2, c % 2
        own = xbf[b, h * T:(h + 1) * T, :]
        oth = xbf[b, (1 - h) * T:(2 - h) * T, :]
        xt = np.ascontiguousarray(np.concatenate([own, oth], axis=0).T)
        in_maps.append({"xt": xt, "wk": wk, "wqw": wqw})
    return in_maps


def _assemble(results):
    out = np.empty((B, S, S), dtype=np.float32)
    for c in range(N_CORES):
        b, h = c // 2, c % 2
        oc = np.asarray(results[c]["o"], dtype=np.float32)
        if h == 1:
            oc = np.concatenate([oc[:, T:], oc[:, :T]], axis=1)
        out[b, h * T:(h + 1) * T, :] = oc
    return out


def kernel(x, Wq, Wk, Ww, _trace_kwargs=None):
    nc = _build_nc()
    in_maps = _make_in_maps(np.asarray(x, dtype=np.float32),
                            np.asarray(Wq, dtype=np.float32),
                            np.asarray(Wk, dtype=np.float32),
                            np.asarray(Ww, dtype=np.float32))
    kw = _trace_kwargs or {}
    res = run_bass_kernel_spmd(nc, in_maps, list(range(N_CORES)), **kw)
    out = _assemble(res.results)
    if _trace_kwargs is not None:
        return out, res
    return out

